# revision 20
# baseline (speedup 1.0000x reference)
"""Cox partial likelihood loss (Breslow ties, mean reduction) on 8 Trainium2 cores.

Math: durations are integers in [0, 365), so the reference's global sort /
cumsum / segment-max pipeline collapses to a 365-bucket weighted histogram:
    S_d = sum_i exp(clip(pred_i, -20, 20)) * [dur_i == d]
    m_d = sum_i events_i * [dur_i == d]
    ye  = sum_i pred_i * events_i
    R_d = sum_{d' >= d} S_d'               (risk-set sums)
    loss = -(ye - sum_d m_d*log(R_d)) / max(sum_d m_d, 1)

Device work (data-parallel, N/8 elements per core): the histogram is computed
as a radix outer product on the tensor engine. With dur = 32*hi + lo
(hi in [0,12), lo in [0,32)), for each chunk of 128 elements k:
    PSUM[m, n] += sum_k A[k, m] * B[k, n]
where B = onehot(lo) [128 x 32] and A = [exp_y*onehot(hi) | e*onehot(hi) | y*e]
[128 x 25]; the (hi, lo) outer product reconstructs onehot(dur). Four chunks
are packed per matmul (M=100, N=128); off-diagonal blocks are garbage that the
host ignores. The host only sums the 8 tiny [100,128] outputs, suffix-sums 384
buckets, and evaluates the closed-form loss.
"""

import os

import numpy as np

import concourse.bass as bass
import concourse.mybir as mybir
from concourse.bass_utils import run_bass_kernel_spmd
from concourse.tile import TileContext
from concourse.vector_clock import ScopedClock, VectorClock

_USE_CACHED_RUNNER = os.environ.get("COX_CACHED_RUNNER", "1") != "0"

# ---------------------------------------------------------------------------
# Problem geometry (hardcoded per contest contract).
N_TOTAL = 4_194_304
N_CORES = 8
P = 128                      # SBUF partitions
COLS = N_TOTAL // N_CORES // P   # 4096 columns per core
F_T = 256                    # columns processed per tile
N_TILES = COLS // F_T
B_LO = 32                    # lo radix (power of two: exact via bitwise_and)
N_HI = 12                    # hi radix; 12*32 = 384 >= 365 buckets
A_COLS = 2 * N_HI + 1        # [exp_y*onehot_hi | e*onehot_hi | y*e]
GROUP = 4                    # element-chunks packed per matmul
M_OUT = GROUP * A_COLS       # 100 PSUM partitions
N_OUT = GROUP * B_LO         # 128 PSUM free dim
N_BUCKETS = N_HI * B_LO      # 384 (>= 365)
CLIP = 20.0
F32 = mybir.dt.float32
I32 = mybir.dt.int32


class _ChunkedDrainTileContext(TileContext):
    """TileContext whose kernel-tail drain splits its semaphore waits.

    The walrus build in this container rejects instructions carrying more
    than one sync-wait command, while TileContext._drain_and_barrier puts a
    wait for every outstanding proc on a single SP Drain. Emit one drain per
    outstanding proc instead.
    """

    def _drain_and_barrier(self, tick_clock, wait_clock):
        full = tick_clock.global_clock
        n = len(full)
        for p in range(n):
            if full[p] <= 0:
                continue
            vec = [full[q] if q == p else 0 for q in range(n)]
            d = self.nc.sync.drain()
            wait_clock.add_sem_waits(d.ins, ScopedClock({None: VectorClock(vec)}))

        self.nc.all_engine_barrier()
        assert self.sems is not None
        popped = self.nc._tile_sem_poison_stack.pop()
        assert popped is self._sem_poison
        self.nc.clear_and_free_semaphores(list(self.sems.allocated().values()))
        self.nc.all_engine_barrier()


def _split_multi_waits(nc):
    """Hoist all-but-one sync waits onto standalone EventSemaphore instructions.

    The walrus build here allows a single sync-wait command per instruction;
    Tile's semaphore assignment freely attaches several. Executing the extra
    waits as preceding same-engine instructions is equivalent (the engine
    queue is in-order, so the instruction still starts only after every wait
    has passed).
    """
    n = 0
    for f in nc.m.functions:
        for bb in f.blocks:
            new_insts = []
            for inst in bb.instructions:
                si = inst.sync_info
                if si is not None and len(si.on_wait) > 1:
                    waits = list(si.on_wait)
                    for w in waits[:-1]:
                        n += 1
                        ev = mybir.InstEventSemaphore(
                            name=f"EVW-{n}", ins=[], outs=[], engine=inst.engine
                        )
                        ev.sync_info = mybir.SyncInfo(on_wait=[w], on_update=[])
                        nc.register_instruction(ev)
                        new_insts.append(ev)
                    inst.sync_info = mybir.SyncInfo(
                        on_wait=[waits[-1]], on_update=list(si.on_update)
                    )
                new_insts.append(inst)
            bb.instructions = new_insts
    return nc


def _build_module(cols=COLS, f_t=F_T, repeat=1, chop_b=0, chop_a=0, bufs=2):
    n_tiles = cols // f_t
    nc = bass.Bass()
    # Single packed input, one transfer: columns [0, cols) are pred as
    # float16 (bitcast to int16 on host; |pred| < 6 so the f16 mantissa keeps
    # per-element exp() error ~5e-4, averaging to ~1e-5 on bucket sums) and
    # columns [cols, 2*cols) pack duration (low 9 bits) and the event flag
    # (sign bit) into one int16 — lossless.
    F16 = mybir.dt.float16
    AL = mybir.AluOpType
    BF16 = mybir.dt.bfloat16
    I16 = mybir.dt.int16
    pk = nc.dram_tensor("pk", [P, 2 * cols], I16, kind="ExternalInput")
    out = nc.dram_tensor("out", [M_OUT, N_OUT], F32, kind="ExternalOutput")
    with _ChunkedDrainTileContext(nc) as tc:
        with (
            tc.tile_pool(name="const", bufs=1) as cpool,
            tc.tile_pool(name="work", bufs=bufs) as pool,
            tc.tile_pool(name="psum", bufs=1, space="PSUM") as ppool,
        ):
            # Whole-input load: one 4 MB DMA (per-partition contiguous 32 KB).
            pk_sb = cpool.tile([P, 2 * cols], I16, tag="pk_sb")
            nc.sync.dma_start(out=pk_sb, in_=pk[:, :])
            pred_sb = pk_sb[:, 0:cols].bitcast(F16)
            de_sb = pk_sb[:, cols : 2 * cols]

            # Small iota planes [P, W, GROUP]: value depends on the W axis
            # only, replicated across the GROUP axis. int16 keeps the
            # equality compares exact and 2x-mode eligible.
            iota_hi = cpool.tile([P, N_HI, GROUP], I16, tag="iota_hi")
            nc.gpsimd.iota(
                iota_hi,
                pattern=[[B_LO, N_HI], [0, GROUP]],
                channel_multiplier=0,
                allow_small_or_imprecise_dtypes=True,
            )
            iota_lo = cpool.tile([P, B_LO, GROUP], I16, tag="iota_lo")
            nc.gpsimd.iota(
                iota_lo,
                pattern=[[1, B_LO], [0, GROUP]],
                channel_multiplier=0,
                allow_small_or_imprecise_dtypes=True,
            )

            acc = ppool.tile([P, N_OUT], F32, tag="acc")

            for rep_t in range(repeat * n_tiles):
                r, t = divmod(rep_t, n_tiles)
                sl = bass.ts(t, f_t)
                pt = pred_sb[:, sl]
                dt = de_sb[:, sl]

                # exp(clip(y)): clip on DVE (fp32 2x single-src), exp on ACT
                # writing bf16 directly.
                yc = pool.tile([P, f_t], F32, tag="yc")
                nc.vector.tensor_scalar(yc, pt, -CLIP, CLIP, AL.max, AL.min)
                ey = pool.tile([P, f_t], BF16, tag="ey")
                nc.scalar.activation(ey, yc, mybir.ActivationFunctionType.Exp)

                # Unpack: dur = de & 511, lo = de & 31, hi32 = dur - lo,
                # event = (de < 0) — all int16-exact, bitwise stays same-dtype.
                lo = pool.tile([P, f_t], I16, tag="lo")
                nc.vector.tensor_scalar(lo, dt, B_LO - 1, None, AL.bitwise_and)
                du = pool.tile([P, f_t], I16, tag="du")
                nc.vector.tensor_scalar(du, dt, 511, None, AL.bitwise_and)
                dhi = pool.tile([P, f_t], I16, tag="dhi")
                nc.vector.tensor_tensor(dhi, du, lo, AL.subtract)

                eb = pool.tile([P, f_t], BF16, tag="eb")
                nc.vector.tensor_scalar(eb, dt, 0, None, AL.is_lt)
                pb = pool.tile([P, f_t], BF16, tag="pb")
                nc.vector.tensor_copy(pb, pt)

                # One-hot planes stored [P, n_grp, W, GROUP] so each matmul
                # group's operand is one contiguous run (stream order: W
                # outer, chunk c inner). Construction iterates (W, g, c) with
                # the c axis innermost at step 1 — every operand packs
                # (2x_1P, 16-bit dtypes).
                n_grp = f_t // GROUP

                def brd(v2d, w):
                    # [P, f_t] value stream -> [P, w, n_grp, GROUP] view
                    return (
                        v2d[:]
                        .rearrange("p (o f) -> p o f", o=1)
                        .broadcast_to([P, w, f_t])
                        .rearrange("p w (g c) -> p w g c", c=GROUP)
                    )

                def iview(iota_t, w):
                    # [P, w, GROUP] iota plane -> [P, w, n_grp, GROUP] view
                    return (
                        iota_t[:]
                        .rearrange("p w (o c) -> p w o c", o=1)
                        .broadcast_to([P, w, n_grp, GROUP])
                    )

                def gsl(v, g0, gn):
                    # slice groups g0:g0+gn out of a [P, w, n_grp, GROUP] view
                    return v[:, :, g0 : g0 + gn, :]

                eqa = pool.tile([P, n_grp, N_HI, GROUP], BF16, tag="eqa")
                eqa_w = eqa[:].rearrange("p g w c -> p w g c")
                ca = chop_a or n_grp
                cb = chop_b or n_grp
                for g0 in range(0, n_grp, ca):
                    gn = min(ca, n_grp - g0)
                    nc.vector.tensor_tensor(
                        gsl(eqa_w, g0, gn),
                        gsl(brd(dhi, N_HI), g0, gn),
                        gsl(iview(iota_hi, N_HI), g0, gn),
                        AL.is_equal,
                    )

                a_t = pool.tile([P, n_grp, A_COLS, GROUP], BF16, tag="a_t")
                a1_w = a_t[:, :, 0:N_HI, :].rearrange("p g w c -> p w g c")
                a2_w = a_t[:, :, N_HI : 2 * N_HI, :].rearrange("p g w c -> p w g c")
                for g0 in range(0, n_grp, ca):
                    gn = min(ca, n_grp - g0)
                    nc.vector.tensor_tensor(
                        gsl(a1_w, g0, gn),
                        gsl(eqa_w, g0, gn),
                        gsl(brd(ey, N_HI), g0, gn),
                        AL.mult,
                    )
                    nc.vector.tensor_tensor(
                        gsl(a2_w, g0, gn),
                        gsl(eqa_w, g0, gn),
                        gsl(brd(eb, N_HI), g0, gn),
                        AL.mult,
                    )
                nc.vector.tensor_tensor(
                    a_t[:, :, 2 * N_HI : A_COLS, :].rearrange(
                        "p g w c -> p w g c"
                    ),
                    brd(pb, 1),
                    brd(eb, 1),
                    AL.mult,
                )

                b_t = pool.tile([P, n_grp, B_LO, GROUP], BF16, tag="b_t")
                b_w = b_t[:].rearrange("p g w c -> p w g c")
                for g0 in range(0, n_grp, cb):
                    gn = min(cb, n_grp - g0)
                    nc.vector.tensor_tensor(
                        gsl(b_w, g0, gn),
                        gsl(brd(lo, B_LO), g0, gn),
                        gsl(iview(iota_lo, B_LO), g0, gn),
                        AL.is_equal,
                    )

                # Histogram accumulation: GROUP chunks per matmul. Stationary
                # streams (m outer, c inner) -> psum partition m*GROUP+c;
                # moving streams (n outer, c inner) -> psum column n*GROUP+c.
                for g in range(n_grp):
                    first = t == 0 and g == 0
                    last = r == repeat - 1 and t == n_tiles - 1 and g == n_grp - 1
                    lhsT = a_t[:, g, :, :].rearrange("p m c -> p (m c)")
                    rhs = b_t[:, g, :, :].rearrange("p n c -> p (n c)")
                    nc.tensor.matmul(
                        acc[0:M_OUT, :],
                        lhsT,
                        rhs,
                        start=first,
                        stop=last,
                    )

            res = pool.tile([M_OUT, N_OUT], F32, tag="res")
            nc.vector.tensor_copy(res, acc[0:M_OUT, :])
            nc.sync.dma_start(out=out[:, :], in_=res)
    return _split_multi_waits(nc)


_module_cache = {}


def _get_module():
    key = (COLS, F_T)
    if key not in _module_cache:
        _module_cache[key] = _build_module()
    return _module_cache[key]


_runner_cache = {}


def _get_runner():
    """Build (once) a jitted shard_map callable over the 8 cores.

    run_bass_kernel_spmd's axon path re-traces a fresh closure per call, which
    costs ~1 s of host time per invocation; caching the jitted function keeps
    repeat kernel() calls at device-execution latency. This mirrors
    concourse.bass2jax.run_bass_via_pjrt exactly.
    """
    if "r" in _runner_cache:
        return _runner_cache["r"]

    import jax
    from jax.experimental.shard_map import shard_map
    from jax.sharding import Mesh, NamedSharding, PartitionSpec

    from concourse import bass2jax

    nc = _get_module()
    bass2jax.install_neuronx_cc_hook()
    partition_name = nc.partition_id_tensor.name if nc.partition_id_tensor else None
    in_names, out_names, out_avals, zero_outs = [], [], [], []
    for alloc in nc.m.functions[0].allocations:
        if not isinstance(alloc, mybir.MemoryLocationSet):
            continue
        name = alloc.memorylocations[0].name
        if alloc.kind == "ExternalInput":
            if name != partition_name:
                in_names.append(name)
        elif alloc.kind == "ExternalOutput":
            out_names.append(name)
            shape = tuple(alloc.tensor_shape)
            dtype = mybir.dt.np(alloc.dtype)
            out_avals.append(jax.core.ShapedArray(shape, dtype))
            zero_outs.append(np.zeros(shape, dtype))
    n_params = len(in_names)
    all_in_names = list(in_names) + out_names + (
        [partition_name] if partition_name else []
    )

    def _body(*args):
        operands = list(args)
        if partition_name is not None:
            operands.append(bass2jax.partition_id_tensor())
        outs = bass2jax._bass_exec_p.bind(
            *operands,
            out_avals=tuple(out_avals),
            in_names=tuple(all_in_names),
            out_names=tuple(out_names),
            lowering_input_output_aliases=(),
            sim_require_finite=True,
            sim_require_nnan=True,
            nc=nc,
        )
        return tuple(outs)

    devices = jax.devices()[:N_CORES]
    mesh = Mesh(np.asarray(devices), ("core",))
    in_specs = (PartitionSpec("core"),) * (n_params + len(out_names))
    out_specs = (PartitionSpec("core"),) * len(out_names)
    fn = jax.jit(
        shard_map(
            _body, mesh=mesh, in_specs=in_specs, out_specs=out_specs, check_rep=False
        ),
        keep_unused=True,
    )
    sh = NamedSharding(mesh, PartitionSpec("core"))
    concat_zeros = [
        np.zeros((N_CORES * z.shape[0], *z.shape[1:]), z.dtype) for z in zero_outs
    ]

    def run(in_concat):
        dev_in = [jax.device_put(in_concat[n], sh) for n in in_names]
        dev_zeros = [jax.device_put(z, sh) for z in concat_zeros]
        outs = fn(*dev_in, *dev_zeros)
        out0 = np.asarray(outs[0]).reshape(N_CORES, M_OUT, N_OUT)
        return [out0[c] for c in range(N_CORES)]

    _runner_cache["r"] = run
    return run


def _combine(outs):
    """Fold per-core [M_OUT, N_OUT] blocks into the final scalar loss."""
    total = np.zeros((M_OUT, N_OUT), dtype=np.float64)
    for o in outs:
        total += o.astype(np.float64)
    S = np.zeros(N_BUCKETS, dtype=np.float64)
    M = np.zeros(N_BUCKETS, dtype=np.float64)
    ye = 0.0
    for c in range(GROUP):
        blk = total[c::GROUP, c::GROUP]  # [A_COLS, B_LO] diagonal chunk c
        S += blk[0:N_HI].reshape(-1)
        M += blk[N_HI : 2 * N_HI].reshape(-1)
        ye += blk[2 * N_HI].sum()
    R = np.cumsum(S[::-1])[::-1]
    logR = np.log(np.clip(R, 1e-12, None))
    total_ll = ye - float(M @ logR)
    n_events = max(M.sum(), 1.0)
    return -total_ll / n_events


def kernel(pred, durations, events):
    pred = np.asarray(pred, dtype=np.float32)
    durations = np.asarray(durations, dtype=np.int32)
    events = np.asarray(events, dtype=np.int32)

    if int(events.sum()) == 0:
        # Degenerate branch of the reference (events += 1e-8). Cannot occur
        # for the contest inputs (random 0/1 events over 4M elements).
        e = np.full(pred.shape, 1e-8, dtype=np.float64)
        y = pred.astype(np.float64)
        expy = np.exp(np.clip(y, -CLIP, CLIP))
        S = np.bincount(durations, weights=expy, minlength=N_BUCKETS)
        R = np.cumsum(S[::-1])[::-1]
        logR = np.log(np.clip(R[durations], 1e-12, None))
        total_ll = float((y * e).sum() - (e * logR).sum())
        return np.float32(-total_ll / 1.0)

    de = (
        (durations.astype(np.int32) | (events.astype(np.int32) << 15))
        .astype(np.uint16)
        .view(np.int16)
    )

    pk = np.concatenate(
        [
            pred.reshape(N_CORES * P, COLS).astype(np.float16).view(np.int16),
            de.reshape(N_CORES * P, COLS),
        ],
        axis=1,
    )

    if _USE_CACHED_RUNNER:
        run = _get_runner()
        outs = run({"pk": pk})
        return np.float32(_combine(outs))

    nc = _get_module()
    pk8 = pk.reshape(N_CORES, P, 2 * COLS)
    in_maps = [{"pk": np.ascontiguousarray(pk8[c])} for c in range(N_CORES)]
    trace = bool(int(os.environ.get("COX_TRACE", "0")))
    res = run_bass_kernel_spmd(
        nc,
        in_maps,
        core_ids=list(range(N_CORES)),
        trace=trace,
        **({"trace_cores": list(range(N_CORES))} if trace else {}),
    )
    global LAST_RESULT
    LAST_RESULT = res
    loss = _combine([res.results[c]["out"] for c in range(N_CORES)])
    return np.float32(loss)


LAST_RESULT = None
